# revision 47
# baseline (speedup 1.0000x reference)
"""Adaptive Computation Time step kernel for 8 TRN2 NeuronCores.

Data-parallel over the batch dim: B=8 rows, one row per core.

Per core (M=4096 positions, H=512 hidden, T=32 tiles of P=128 rows):
  - unpack:   h_full[m] = h[rank[m]] with rank = clip(cumsum(run)-1, 0)
              (implemented as 32 indirect-DMA row gathers of 2KB rows)
  - halting:  p = sigmoid(h_full @ W + b) * run
  - masks:    acc_new = acc + p; mc = (acc_new < 0.99) & run; me = run & ~mc
              p_eff = p*mc + (1-acc)*me; rem_new = rem + (1-acc)*me
  - blend:    w_new = (h_full - w)*p_eff + w
  - pack:     h_out[c_excl[m]] = h_full[m] for mc[m]  (indirect-DMA row
              scatter; skipped rows stay zero because the runtime hands the
              NEFF pre-zeroed output buffers; the scatter is split over 8
              destination tensors to avoid false WAW serialization, merged
              on the host by summation - destinations are disjoint).

All [M,1] per-position math is done in "layout L": an SBUF tile [128, 32]
where element (j, t) is position m = 128*t + j.  Column t of that tile is
exactly tile t's per-partition scalar, so per-tile ops slice columns.
Cross-partition cumsums are triangular-matrix matmuls on the TensorEngine;
cross-column (tile) offsets use a free-dim scan.
"""

import numpy as np

import concourse.bass as bass
import concourse.bacc as bacc
import concourse.tile as tile
from concourse import mybir
from concourse.bass_utils import run_bass_kernel_spmd

P = 128          # SBUF partitions / rows per tile
T = 32           # tiles per core
M = P * T        # 4096 positions per core
H = 512          # hidden dim
B = 8            # batch == cores
NSPLIT = 8       # h_out scatter destination split
CH = 8           # tiles per small-math chunk
THRESHOLD = 0.99
BIG = 65536.0    # f32-exact OOB marker (> M-1 so bounds check skips it)

F32 = mybir.dt.float32
I32 = mybir.dt.int32
Alu = mybir.AluOpType


def _build_nc(no_gather=False, no_scatter=False, w_preload=False,
              ch=CH, nsplit=NSPLIT, gather_skip=False, chunks=(16, 16),
              nbufs=None):
    nc = bacc.Bacc("TRN2", target_bir_lowering=False, debug=True)

    h_d = nc.declare_dram_parameter("h", [M, H], F32, isOutput=False)
    w_d = nc.declare_dram_parameter("w", [M, H], F32, isOutput=False)
    # columns 0:T = runL, T:2T = accL, 2T:3T = remL (layout L each)
    smallL_d = nc.declare_dram_parameter("smallL", [P, 3 * T], F32,
                                         isOutput=False)
    # host-precomputed constants: [triu_incl | triu_excl | ident] and the
    # halting weights pre-broadcast across partitions, negated bias col
    consts_d = nc.declare_dram_parameter("consts", [P, 3 * P], F32,
                                         isOutput=False)
    wb_d = nc.declare_dram_parameter("wb", [P, H], F32, isOutput=False)
    negb_d = nc.declare_dram_parameter("negb", [P, 1], F32, isOutput=False)

    hout_d = [
        nc.declare_dram_parameter(f"hout{k}", [M, H], F32, isOutput=True)
        for k in range(NSPLIT)
    ]
    wnew_d = nc.declare_dram_parameter("wnew", [M, H], F32, isOutput=True)
    # columns 0:T = accnewL, T:2T = remnewL
    outL_d = nc.declare_dram_parameter("outL", [P, 2 * T], F32, isOutput=True)

    with tile.TileContext(nc) as tc:
        with (
            tc.tile_pool(name="consts", bufs=1) as consts,
            tc.tile_pool(name="Lbuf", bufs=1) as Lb,
            tc.tile_pool(name="hbuf", bufs=1) as hbuf,
            tc.tile_pool(name="diags", bufs=(nbufs or {}).get("diags", 4)) as diags,
            tc.tile_pool(name="outtiles", bufs=(nbufs or {}).get("outtiles", 4)) as outtiles,
            tc.tile_pool(name="scratch", bufs=(nbufs or {}).get("scratch", 3)) as scratch,
            tc.tile_pool(name="incl_pool", bufs=2) as incl_pool,
            tc.tile_pool(name="psum", bufs=2, space="PSUM") as psum,
            tc.tile_pool(name="pblend", bufs=4, space="PSUM") as pblend,
        ):
            # ---- big streaming loads first so DMA is busy from t=0 ----
            h_sb = hbuf.tile([P, T, H], F32)  # gathered rows stay resident
            if gather_skip:
                nc.vector.memset(h_sb[:], 0.0)
            w_sb = hbuf.tile([P, T, H], F32, tag="w_sb")
            w_dr = w_d[:]
            if w_preload:
                for t in range(T):
                    nc.sync.dma_start(out=w_sb[:, t, :],
                                      in_=w_dr[t * P:(t + 1) * P, :])

            # ---- constants (host-precomputed, one DMA each) ----
            cmat = consts.tile([P, 3 * P], F32)
            nc.sync.dma_start(out=cmat[:], in_=consts_d[:])
            triu_incl = cmat[:, 0:P]        # [p, i] = 1 if p <= i
            triu_excl = cmat[:, P:2 * P]    # [p, i] = 1 if p < i
            ident = cmat[:, 2 * P:3 * P]
            Wb = consts.tile([P, H], F32)
            nc.sync.dma_start(out=Wb[:], in_=wb_d[:])
            negb = consts.tile([P, 1], F32)
            nc.sync.dma_start(out=negb[:], in_=negb_d[:])
            ones_col = consts.tile([P, 1], F32)
            nc.vector.memset(ones_col[:], 1.0)
            zeros_row = consts.tile([1, T], F32)
            nc.vector.memset(zeros_row[:], 0.0)

            # ---- small inputs in layout L (one DMA) ----
            smallL = Lb.tile([P, 3 * T], F32)
            nc.sync.dma_start(out=smallL[:], in_=smallL_d[:])
            runL = smallL[:, 0:T]
            accL = smallL[:, T:2 * T]
            remL = smallL[:, 2 * T:3 * T]

            # ---- rank = clip(cumsum(run) - 1, 0), in layout L ----
            ps_run = psum.tile([P, T], F32, space="PSUM", tag="pscan")
            nc.tensor.matmul(ps_run[:], lhsT=triu_incl[:], rhs=runL[:],
                             start=True, stop=True)
            ps_runtot = psum.tile([1, T], F32, space="PSUM", tag="ptot")
            nc.tensor.matmul(ps_runtot[:], lhsT=ones_col[:], rhs=runL[:],
                             start=True, stop=True)
            tot_run = Lb.tile([1, T], F32)
            nc.vector.tensor_copy(out=tot_run[:], in_=ps_runtot[:])
            incl_run = Lb.tile([1, T], F32)
            nc.vector.tensor_tensor_scan(
                incl_run[:], tot_run[:], zeros_row[:], 0.0, Alu.add, Alu.add)
            excl_run = Lb.tile([1, T], F32)
            nc.vector.tensor_tensor(
                out=excl_run[:], in0=incl_run[:], in1=tot_run[:], op=Alu.subtract)
            exclb_run = Lb.tile([P, T], F32)
            nc.gpsimd.partition_broadcast(exclb_run[:], excl_run[:])
            cums = Lb.tile([P, T], F32)
            nc.vector.tensor_tensor(
                out=cums[:], in0=ps_run[:], in1=exclb_run[:], op=Alu.add)
            rankf = Lb.tile([P, T], F32)
            nc.vector.tensor_scalar(
                out=rankf[:], in0=cums[:], scalar1=-1.0, scalar2=0.0,
                op0=Alu.add, op1=Alu.max)
            rank_i = Lb.tile([P, T], I32)
            if gather_skip:
                # skip gathers for non-running rows (their h_sb slots stay 0
                # from the memset): idx = run ? rank : BIG
                gskipf = Lb.tile([P, T], F32)
                nc.vector.tensor_scalar_add(gskipf[:], rankf[:], -BIG)
                nc.vector.tensor_tensor(
                    out=gskipf[:], in0=gskipf[:], in1=runL[:], op=Alu.mult)
                nc.vector.tensor_scalar_add(gskipf[:], gskipf[:], BIG)
                nc.vector.tensor_copy(out=rank_i[:], in_=gskipf[:])
            else:
                nc.vector.tensor_copy(out=rank_i[:], in_=rankf[:])

            # ---- persistent L tensors written along the pipeline ----
            logitL = Lb.tile([P, T], F32)
            pexpL = Lb.tile([P, T], F32)
            sigL = Lb.tile([P, T], F32)
            pL = Lb.tile([P, T], F32)
            outL = Lb.tile([P, 2 * T], F32)
            accnewL = outL[:, 0:T]
            remnewL = outL[:, T:2 * T]
            mcL = Lb.tile([P, T], F32)
            meL = Lb.tile([P, T], F32)
            omaL = Lb.tile([P, T], F32)     # 1 - acc
            t2L = Lb.tile([P, T], F32)      # (1-acc)*me
            t3L = Lb.tile([P, T], F32)      # p*mc
            peL = Lb.tile([P, T], F32)
            ompeL = Lb.tile([P, T], F32)    # 1 - p_eff
            cexcl = Lb.tile([P, T], F32)    # global exclusive cumsum of mc
            idxf = Lb.tile([P, T], F32)
            idx_i = Lb.tile([P, T], I32)
            zeros_chunk = consts.tile([1, max(chunks) if chunks else ch], F32)
            nc.vector.memset(zeros_chunk[:], 0.0)
            prev_incl = None

            # ---- main pipeline ----
            chunk_list = list(chunks) if chunks else [ch] * (T // ch)
            assert sum(chunk_list) == T
            chunk_starts = [sum(chunk_list[:i]) for i in range(len(chunk_list))]
            for c in range(len(chunk_list)):
                c0, cw = chunk_starts[c], chunk_list[c]
                cols = slice(c0, c0 + cw)
                for t in range(c0, c0 + cw):
                    if no_gather:
                        nc.sync.dma_start(
                            out=h_sb[:, t, :], in_=h_d[t * P:(t + 1) * P, :])
                    else:
                        gkw = (dict(bounds_check=M - 1, oob_is_err=False)
                               if gather_skip else {})
                        nc.gpsimd.indirect_dma_start(
                            out=h_sb[:, t, :], out_offset=None, in_=h_d[:],
                            in_offset=bass.IndirectOffsetOnAxis(
                                ap=rank_i[:, t:t + 1], axis=0), **gkw)
                    prod = scratch.tile([P, H], F32)
                    nc.vector.tensor_tensor(
                        out=prod[:], in0=h_sb[:, t, :], in1=Wb[:], op=Alu.mult)
                    dump = scratch.tile([P, H], F32, tag="dump")
                    nc.scalar.activation(
                        out=dump[:], in_=prod[:],
                        func=mybir.ActivationFunctionType.Copy,
                        accum_out=logitL[:, t:t + 1])

                # per-chunk small math on [P, CH] column slices
                nc.scalar.activation(
                    out=pexpL[:, cols], in_=logitL[:, cols],
                    func=mybir.ActivationFunctionType.Exp,
                    bias=negb[:], scale=-1.0)
                nc.vector.tensor_scalar_add(sigL[:, cols], pexpL[:, cols], 1.0)
                nc.vector.reciprocal(sigL[:, cols], sigL[:, cols])
                nc.vector.tensor_tensor(
                    out=pL[:, cols], in0=sigL[:, cols], in1=runL[:, cols],
                    op=Alu.mult)
                nc.vector.tensor_tensor(
                    out=accnewL[:, cols], in0=accL[:, cols], in1=pL[:, cols],
                    op=Alu.add)
                nc.vector.tensor_scalar(
                    out=mcL[:, cols], in0=accnewL[:, cols],
                    scalar1=THRESHOLD, scalar2=None, op0=Alu.is_lt)
                nc.vector.tensor_tensor(
                    out=mcL[:, cols], in0=mcL[:, cols], in1=runL[:, cols],
                    op=Alu.mult)
                nc.vector.tensor_tensor(
                    out=meL[:, cols], in0=runL[:, cols], in1=mcL[:, cols],
                    op=Alu.subtract)
                nc.vector.tensor_scalar(
                    out=omaL[:, cols], in0=accL[:, cols], scalar1=-1.0,
                    scalar2=1.0, op0=Alu.mult, op1=Alu.add)
                nc.vector.tensor_tensor(
                    out=t2L[:, cols], in0=omaL[:, cols], in1=meL[:, cols],
                    op=Alu.mult)
                nc.vector.tensor_tensor(
                    out=remnewL[:, cols], in0=remL[:, cols], in1=t2L[:, cols],
                    op=Alu.add)
                nc.vector.tensor_tensor(
                    out=t3L[:, cols], in0=pL[:, cols], in1=mcL[:, cols],
                    op=Alu.mult)
                nc.vector.tensor_tensor(
                    out=peL[:, cols], in0=t2L[:, cols], in1=t3L[:, cols],
                    op=Alu.add)
                nc.vector.tensor_scalar(
                    out=ompeL[:, cols], in0=peL[:, cols], scalar1=-1.0,
                    scalar2=1.0, op0=Alu.mult, op1=Alu.add)

                # per-chunk pack indices: global exclusive cumsum of mc.
                # within-column exclusive scan + per-column offsets, with the
                # running total carried across chunks via the scan's initial.
                ps_c = psum.tile([P, cw], F32, space="PSUM", tag="pscan")
                nc.tensor.matmul(ps_c[:], lhsT=triu_excl[:],
                                 rhs=mcL[:, cols], start=True, stop=True)
                ps_ct = psum.tile([1, cw], F32, space="PSUM", tag="ptot")
                nc.tensor.matmul(ps_ct[:], lhsT=ones_col[:],
                                 rhs=mcL[:, cols], start=True, stop=True)
                ct = Lb.tile([1, cw], F32, tag="ct")
                nc.vector.tensor_copy(out=ct[:], in_=ps_ct[:])
                incl = incl_pool.tile([1, cw], F32, tag="incl")
                init = 0.0 if c == 0 else prev_incl[:, prev_cw - 1:prev_cw]
                nc.vector.tensor_tensor_scan(
                    incl[:], ct[:], zeros_chunk[:, :cw], init, Alu.add, Alu.add)
                prev_incl = incl; prev_cw = cw
                excl_c = Lb.tile([1, cw], F32, tag="excl_c")
                nc.vector.tensor_tensor(
                    out=excl_c[:], in0=incl[:], in1=ct[:], op=Alu.subtract)
                exclb_c = Lb.tile([P, cw], F32, tag="exclb_c")
                nc.gpsimd.partition_broadcast(exclb_c[:], excl_c[:])
                nc.vector.tensor_tensor(
                    out=cexcl[:, cols], in0=ps_c[:], in1=exclb_c[:], op=Alu.add)
                # idx = mc ? cexcl : BIG   ==  (cexcl - BIG)*mc + BIG
                nc.vector.tensor_scalar_add(
                    idxf[:, cols], cexcl[:, cols], -BIG)
                nc.vector.tensor_tensor(
                    out=idxf[:, cols], in0=idxf[:, cols], in1=mcL[:, cols],
                    op=Alu.mult)
                nc.vector.tensor_scalar_add(idxf[:, cols], idxf[:, cols], BIG)
                nc.vector.tensor_copy(out=idx_i[:, cols], in_=idxf[:, cols])

                for t in range(c0, c0 + cw):
                    w_t = w_sb[:, t, :]
                    if not w_preload:
                        nc.sync.dma_start(out=w_t,
                                          in_=w_dr[t * P:(t + 1) * P, :])
                    # wnew = diag(pe) @ h_full + diag(1-pe) @ w
                    d1 = diags.tile([P, P], F32, tag="d1")
                    nc.vector.tensor_scalar(
                        out=d1[:], in0=ident[:], scalar1=peL[:, t:t + 1],
                        scalar2=None, op0=Alu.mult)
                    d2 = diags.tile([P, P], F32, tag="d2")
                    nc.vector.tensor_scalar(
                        out=d2[:], in0=ident[:], scalar1=ompeL[:, t:t + 1],
                        scalar2=None, op0=Alu.mult)
                    ps_w = pblend.tile([P, H], F32, space="PSUM")
                    nc.tensor.matmul(ps_w[:], lhsT=d1[:], rhs=h_sb[:, t, :],
                                     start=True, stop=False)
                    nc.tensor.matmul(ps_w[:], lhsT=d2[:], rhs=w_t,
                                     start=False, stop=True)
                    wn_t = outtiles.tile([P, H], F32)
                    nc.scalar.copy(out=wn_t[:], in_=ps_w[:])
                    nc.sync.dma_start(
                        out=wnew_d[t * P:(t + 1) * P, :], in_=wn_t[:])
                    if no_scatter:
                        nc.sync.dma_start(
                            out=hout_d[t % nsplit][t * P:(t + 1) * P, :],
                            in_=h_sb[:, t, :])
                    else:
                        nc.gpsimd.indirect_dma_start(
                            out=hout_d[t % nsplit][:],
                            out_offset=bass.IndirectOffsetOnAxis(
                                ap=idx_i[:, t:t + 1], axis=0),
                            in_=h_sb[:, t, :], in_offset=None,
                            bounds_check=M - 1, oob_is_err=False)

            nc.sync.dma_start(out=outL_d[:], in_=outL[:])

    nc.compile()
    return nc


_NC = None


def _get_nc():
    global _NC
    if _NC is None:
        _NC = _build_nc()
    return _NC


def _to_L(a):
    """[M] position-order vector -> [P, T] layout-L (L[j, t] = a[128t + j])."""
    return np.ascontiguousarray(a.reshape(T, P).T)


def _from_L(L):
    """[P, T] layout-L -> [M] position-order vector."""
    return np.ascontiguousarray(L.T).reshape(M)


def _run(h, run, acc_p, weighted_h, remainders, W, b, trace=False):
    h = np.ascontiguousarray(np.asarray(h, np.float32))
    runf = np.asarray(run).astype(np.float32)
    acc_p = np.asarray(acc_p, np.float32)
    weighted_h = np.ascontiguousarray(np.asarray(weighted_h, np.float32))
    remainders = np.asarray(remainders, np.float32)
    wb = np.ascontiguousarray(
        np.broadcast_to(np.asarray(W, np.float32).reshape(1, H), (P, H)))
    negb = np.ascontiguousarray(
        np.broadcast_to(-np.asarray(b, np.float32).reshape(1, 1), (P, 1)))
    ii = np.arange(P)
    consts_m = np.concatenate([
        (ii[:, None] <= ii[None, :]).astype(np.float32),
        (ii[:, None] < ii[None, :]).astype(np.float32),
        np.eye(P, dtype=np.float32)], axis=1)
    consts_m = np.ascontiguousarray(consts_m)

    in_maps = []
    for i in range(B):
        smallL = np.concatenate(
            [_to_L(runf[i, :, 0]), _to_L(acc_p[i, :, 0]),
             _to_L(remainders[i, :, 0])], axis=1)
        in_maps.append({
            "h": h[i],
            "w": weighted_h[i],
            "smallL": np.ascontiguousarray(smallL),
            "consts": consts_m,
            "wb": wb,
            "negb": negb,
        })

    nc = _get_nc()
    res = run_bass_kernel_spmd(
        nc, in_maps, core_ids=list(range(B)), trace=trace)

    h_out = np.empty((B, M, H), np.float32)
    wnew = np.empty((B, M, H), np.float32)
    accnew = np.empty((B, M, 1), np.float32)
    remnew = np.empty((B, M, 1), np.float32)
    for i in range(B):
        r = res.results[i]
        ho = r["hout0"].copy()
        for k in range(1, NSPLIT):
            ho += r[f"hout{k}"]
        h_out[i] = ho
        wnew[i] = r["wnew"]
        accnew[i, :, 0] = _from_L(r["outL"][:, 0:T])
        remnew[i, :, 0] = _from_L(r["outL"][:, T:2 * T])
    return (h_out, wnew, accnew, remnew), res


def kernel(h, run, acc_p, weighted_h, remainders, W, b):
    out, _ = _run(h, run, acc_p, weighted_h, remainders, W, b, trace=False)
    return out
